# revision 9
# baseline (speedup 1.0000x reference)
"""MoE post-processing MLP kernel for Trainium2 (8 NeuronCores).

Strategy: expert-parallel sharding. Each core is assigned one chunk of
samples routed to a single expert (K=8 experts ~= 8 cores for uniform
routing). The host gathers/permutes samples by expert, the device runs a
dense 3-layer MLP with the positional encoding (sin/cos) computed
on-chip, and the host scatters results back to original order.

Device kernel (per core, C=9216 samples as 9 iterations of 1024):
  - samples pair-packed: tile [78, 512] = two 39-row blocks (feat32 +
    pos3 + view3 + ones) for 2x512 samples
  - u = Rdd^T @ fpv   (posenc angles / 2pi + phase, on TensorE)
  - r = round(u) via magic-constant two-op tensor_scalar (DVE)
  - v = u - r         (DVE; range reduction mod 1)
  - s36 = sin(2pi*v) on ScalarE (LUT valid in [-pi, pi])
  - h0 = relu(W0a^T@fpv + W0s^T@s36 + b0); h1 = relu(W1^T@h0 + b1)
  - y = W2^T@h1 + b2  (block-diagonal weights process both halves)
All matmuls run in float32r (full-rate fp32 mode on the PE). Input
DMAs are all prefetched on SP; output DMAs issue from GpSimd so the
two directions never serialize on one DGE queue.
"""

import numpy as np

K = 8
WID = 64
D = 32
NT = 512            # matmul moving free dim (one PSUM bank of fp32)
NITER = 9           # iterations per invocation
C = NITER * 2 * NT  # 9216 samples per core-chunk
CMAGIC = 12582912.0  # 1.5 * 2**23, round-to-nearest magic constant

# Wall column layout: [Rdd 72 | W0add 128 | W0sdd 128 | W1dd 128 |
#                      W2dd 64 | b0 1 | b1 1 | b2 1]
_COLS = 72 + 128 + 128 + 128 + 64 + 3

# W0 row indices (DIN=74 layout: feat 0:32, posenc(pos,2) 32:47,
# posenc(view,4) 47:74) for the identity part and the sin part.
_W0A_ROWS = list(range(32)) + [32, 33, 34] + [47, 48, 49]
_W0S_ROWS = (list(range(35, 41)) + list(range(50, 62))
             + list(range(41, 47)) + list(range(62, 74)))

_PREP = None  # compiled Bass program, built once per process
_LAST_IN_MAPS = None  # stashed for external profiling harnesses


def _build_R():
    """R' [7, 36]: u = (scale*x + phase)/(2pi); rows = [p0..2, v0..2, 1]."""
    Rp = np.zeros((7, 36), np.float32)
    col = 0
    for phase in range(2):
        for base, scales in ((0, [1.0, 2.0]), (3, [1.0, 2.0, 4.0, 8.0])):
            for m in scales:
                for c in range(3):
                    Rp[base + c, col] = m / (2 * np.pi)
                    Rp[6, col] = 0.25 * phase
                    col += 1
    return Rp


def _build_program():
    import concourse.bacc as bacc
    import concourse.mybir as mybir
    from concourse.tile import TileContext

    F32, F32R = mybir.dt.float32, mybir.dt.float32r
    AF = mybir.ActivationFunctionType
    ALU = mybir.AluOpType

    nc = bacc.Bacc("TRN2", target_bir_lowering=False, debug=False,
                   num_devices=8)

    fpv_d = nc.dram_tensor("fpv", [78, C // 2], F32R,
                           kind="ExternalInput").ap()
    wall_d = nc.dram_tensor("wall", [128, _COLS], F32R,
                            kind="ExternalInput").ap()
    y_d = nc.dram_tensor("y", [64, C // 2], F32, kind="ExternalOutput").ap()

    with TileContext(nc) as tc:
        with (tc.tile_pool(name="w", bufs=1) as wp,
              tc.tile_pool(name="fp", bufs=NITER) as fpool,
              tc.tile_pool(name="io", bufs=3) as io,
              tc.tile_pool(name="ps", bufs=2, space="PSUM") as ps):
            wall = wp.tile([128, _COLS], F32R)
            nc.sync.dma_start(out=wall[:], in_=wall_d[:])
            Rt = wall[0:78, 0:72]
            W0at = wall[0:78, 72:200]
            W0st = wall[0:72, 200:328]
            W1t = wall[0:128, 328:456]
            W2t = wall[0:128, 456:520]
            b0t = wall[0:128, 520:521].bitcast(F32)
            b1t = wall[0:128, 521:522].bitcast(F32)
            b2t = wall[0:64, 522:523].bitcast(F32)

            # prefetch every input tile on SP up front
            fpvts = []
            for i in range(NITER):
                fpvt = fpool.tile([78, NT], F32R)
                nc.sync.dma_start(out=fpvt[:],
                                  in_=fpv_d[:, i * NT:(i + 1) * NT])
                fpvts.append(fpvt)

            # PE warm-up during the DMA fill: ~3.5us of dummy matmuls on
            # the weight tile flips the HAM clock gate to full rate before
            # the real stream begins. Results are overwritten (start=True
            # in the loop below re-initializes each PSUM tile).
            for _ in range(4):
                upw = ps.tile([72, NT], F32, tag="up")
                nc.tensor.matmul(out=upw[:], lhsT=wall[0:78, 0:72],
                                 rhs=wall[0:78, 0:NT], start=True, stop=True)
                nc.tensor.matmul(out=upw[:], lhsT=wall[0:78, 0:72],
                                 rhs=wall[0:78, 0:NT], start=True, stop=True)

            for i in range(NITER):
                fpvt = fpvts[i]
                up = ps.tile([72, NT], F32)
                nc.tensor.matmul(out=up[:], lhsT=Rt, rhs=fpvt[:],
                                 start=True, stop=True)
                rt = io.tile([72, NT], F32)
                nc.vector.tensor_scalar(out=rt[:], in0=up[:], scalar1=CMAGIC,
                                        scalar2=CMAGIC, op0=ALU.add,
                                        op1=ALU.subtract)
                vt = io.tile([72, NT], F32)
                nc.vector.tensor_tensor(out=vt[:], in0=up[:], in1=rt[:],
                                        op=ALU.subtract)
                s36t = io.tile([72, NT], F32R)
                nc.scalar.activation(s36t[:], vt[:], AF.Sin, bias=0.0,
                                     scale=float(2 * np.pi))

                h0p = ps.tile([128, NT], F32)
                nc.tensor.matmul(out=h0p[:], lhsT=W0at, rhs=fpvt[:],
                                 start=True, stop=False)
                nc.tensor.matmul(out=h0p[:], lhsT=W0st, rhs=s36t[:],
                                 start=False, stop=True)
                h0t = io.tile([128, NT], F32R)
                nc.scalar.activation(h0t[:], h0p[:], AF.Relu, bias=b0t,
                                     scale=1.0)

                h1p = ps.tile([128, NT], F32)
                nc.tensor.matmul(out=h1p[:], lhsT=W1t, rhs=h0t[:],
                                 start=True, stop=True)
                h1t = io.tile([128, NT], F32R)
                nc.scalar.activation(h1t[:], h1p[:], AF.Relu, bias=b1t,
                                     scale=1.0)

                yp = ps.tile([64, NT], F32)
                nc.tensor.matmul(out=yp[:], lhsT=W2t, rhs=h1t[:],
                                 start=True, stop=True)
                yt = io.tile([64, NT], F32)
                nc.vector.tensor_scalar(out=yt[:], in0=yp[:], scalar1=b2t,
                                        scalar2=None, op0=ALU.add)
                nc.sync.dma_start(out=y_d[:, i * NT:(i + 1) * NT],
                                  in_=yt[:])

    nc.compile()
    return nc


def _get_program():
    global _PREP
    if _PREP is None:
        _PREP = _build_program()
    return _PREP


def _pack_weights(W0, b0, W1, b1, W2, b2):
    """Per-expert consolidated [128, _COLS] device weight array."""
    W0a = np.zeros((39, 64), np.float32)
    W0a[0:38] = W0[_W0A_ROWS]
    W0s = W0[_W0S_ROWS].astype(np.float32)

    Rp = _build_R()
    wall = np.zeros((128, _COLS), np.float32)
    wall[32:39, 0:36] = Rp          # Rdd block A
    wall[71:78, 36:72] = Rp         # Rdd block B
    wall[0:39, 72:136] = W0a        # W0add block A
    wall[39:78, 136:200] = W0a      # W0add block B
    wall[0:36, 200:264] = W0s       # W0sdd block A
    wall[36:72, 264:328] = W0s      # W0sdd block B
    wall[0:64, 328:392] = W1        # W1dd block A
    wall[64:128, 392:456] = W1      # W1dd block B
    wall[0:64, 456:488] = W2        # W2dd block A
    wall[64:128, 488:520] = W2      # W2dd block B
    wall[0:128, 520] = np.concatenate([b0, b0])
    wall[0:128, 521] = np.concatenate([b1, b1])
    wall[0:64, 522] = np.concatenate([b2, b2])
    return wall


def kernel(idxs, positions, viewdirs, features, W0, b0, W1, b1, W2, b2):
    from concourse.bass_utils import run_bass_kernel_spmd

    N = idxs.shape[0]
    idx = idxs.reshape(-1).astype(np.int64)
    out = np.zeros((N, D), np.float32)

    # Route: list of (expert, sample-index-array) chunks of <= C samples.
    chunks = []
    for k in range(K):
        sel = np.nonzero(idx == k)[0]
        for lo in range(0, len(sel), C):
            chunks.append((k, sel[lo:lo + C]))

    walls = [_pack_weights(W0[k], b0[k], W1[k], b1[k], W2[k], b2[k])
             for k in range(K)]

    nc = _get_program()
    zero_in = None
    for inv in range(0, len(chunks), 8):
        batch = chunks[inv:inv + 8]
        in_maps = []
        for ci in range(8):
            if ci < len(batch):
                k, sel = batch[ci]
                n = len(sel)
                fpv39 = np.zeros((39, C), np.float32)
                fpv39[0:32, :n] = features[sel].T
                fpv39[32:35, :n] = positions[sel].T
                fpv39[35:38, :n] = viewdirs[sel].T
                fpv39[38, :] = 1.0
                fpv78 = np.concatenate(
                    [fpv39.reshape(39, NITER, 2, NT)[:, :, 0],
                     fpv39.reshape(39, NITER, 2, NT)[:, :, 1]],
                    axis=0).reshape(78, C // 2)
                in_maps.append({"fpv": np.ascontiguousarray(fpv78),
                                "wall": walls[k]})
            else:
                if zero_in is None:
                    zero_in = {"fpv": np.zeros((78, C // 2), np.float32),
                               "wall": walls[0]}
                in_maps.append(zero_in)
        global _LAST_IN_MAPS
        _LAST_IN_MAPS = in_maps
        res = None
        for attempt in range(3):
            try:
                res = run_bass_kernel_spmd(nc, in_maps,
                                           core_ids=list(range(8)))
                break
            except Exception:
                if attempt == 2:
                    raise
        assert res is not None
        for ci, (k, sel) in enumerate(batch):
            y64 = res.results[ci]["y"]                   # [64, C//2]
            y32 = np.stack([y64[0:32].reshape(D, NITER, NT),
                            y64[32:64].reshape(D, NITER, NT)],
                           axis=2).reshape(D, C)
            out[sel] = y32[:, :len(sel)].T
    return out


# revision 12
# speedup vs baseline: 1.0824x; 1.0824x over previous
"""MoE post-processing MLP kernel for Trainium2 (8 NeuronCores).

Strategy: expert-parallel sharding. Each core is assigned one chunk of
samples routed to a single expert (K=8 experts ~= 8 cores for uniform
routing). The host gathers/permutes samples by expert, the device runs a
dense 3-layer MLP with the positional encoding (sin/cos) computed
on-chip, and the host scatters results back to original order.

Device kernel (per core, C=9216 samples as 9 iterations of 1024):
  - samples pair-packed: tile [78, 512] = two 39-row blocks (feat32 +
    pos3 + view3 + ones) for 2x512 samples
  - u = Rdd^T @ fpv   (posenc angles / 2pi + phase, on TensorE)
  - r = round(u) via magic-constant two-op tensor_scalar (DVE)
  - v = u - r         (DVE; range reduction mod 1)
  - s36 = sin(2pi*v) on ScalarE (LUT valid in [-pi, pi])
  - h0 = relu(W0a^T@fpv + W0s^T@s36 + b0); h1 = relu(W1^T@h0 + b1)
  - y = W2^T@h1 + b2  (block-diagonal weights process both halves)
All matmuls run in float32r (full-rate fp32 mode on the PE). Input
DMAs are all prefetched on SP; output DMAs issue from GpSimd so the
two directions never serialize on one DGE queue.
"""

import numpy as np

K = 8
WID = 64
D = 32
NT = 512            # matmul moving free dim (one PSUM bank of fp32)
NITER = 9           # iterations per invocation
C = NITER * 2 * NT  # 9216 samples per core-chunk
CMAGIC = 12582912.0  # 1.5 * 2**23, round-to-nearest magic constant

# Wall column layout: [Rdd 72 | W0add 128 | W0sdd 128 | W1dd 128 |
#                      W2dd 64 | b0 1 | b1 1 | b2 1]
_COLS = 72 + 128 + 128 + 128 + 64 + 3

# W0 row indices (DIN=74 layout: feat 0:32, posenc(pos,2) 32:47,
# posenc(view,4) 47:74) for the identity part and the sin part.
_W0A_ROWS = list(range(32)) + [32, 33, 34] + [47, 48, 49]
_W0S_ROWS = (list(range(35, 41)) + list(range(50, 62))
             + list(range(41, 47)) + list(range(62, 74)))

_PREP = None  # compiled Bass program, built once per process
_LAST_IN_MAPS = None  # stashed for external profiling harnesses


def _build_R():
    """R' [7, 36]: u = (scale*x + phase)/(2pi); rows = [p0..2, v0..2, 1]."""
    Rp = np.zeros((7, 36), np.float32)
    col = 0
    for phase in range(2):
        for base, scales in ((0, [1.0, 2.0]), (3, [1.0, 2.0, 4.0, 8.0])):
            for m in scales:
                for c in range(3):
                    Rp[base + c, col] = m / (2 * np.pi)
                    Rp[6, col] = 0.25 * phase
                    col += 1
    return Rp


def _build_program():
    import concourse.bacc as bacc
    import concourse.mybir as mybir
    from concourse.tile import TileContext

    F32, F32R = mybir.dt.float32, mybir.dt.float32r
    AF = mybir.ActivationFunctionType
    ALU = mybir.AluOpType

    nc = bacc.Bacc("TRN2", target_bir_lowering=False, debug=False,
                   num_devices=8)

    fpv_d = nc.dram_tensor("fpv", [78, C // 2], F32R,
                           kind="ExternalInput").ap()
    wall_d = nc.dram_tensor("wall", [128, _COLS], F32R,
                            kind="ExternalInput").ap()
    y_d = nc.dram_tensor("y", [64, C // 2], F32, kind="ExternalOutput").ap()

    with TileContext(nc) as tc:
        with (tc.tile_pool(name="w", bufs=1) as wp,
              tc.tile_pool(name="fp", bufs=NITER) as fpool,
              tc.tile_pool(name="io", bufs=3) as io,
              tc.tile_pool(name="ps", bufs=1, space="PSUM") as ps):
            wall = wp.tile([128, _COLS], F32R)
            nc.sync.dma_start(out=wall[:], in_=wall_d[:])
            Rt = wall[0:78, 0:72]
            W0at = wall[0:78, 72:200]
            W0st = wall[0:72, 200:328]
            W1t = wall[0:128, 328:456]
            W2t = wall[0:128, 456:520]
            b0t = wall[0:128, 520:521].bitcast(F32)
            b1t = wall[0:128, 521:522].bitcast(F32)
            b2t = wall[0:64, 522:523].bitcast(F32)

            # prefetch every input tile on SP up front
            fpvts = []
            for i in range(NITER):
                fpvt = fpool.tile([78, NT], F32R)
                nc.sync.dma_start(out=fpvt[:],
                                  in_=fpv_d[:, i * NT:(i + 1) * NT])
                fpvts.append(fpvt)

            # super-iterations: 2 sample-pairs (2048 samples) share one
            # [*, 1024] PSUM tile per layer; matmuls write 512-wide halves,
            # elementwise ops process the full 1024-wide tile in one shot.
            sup = []
            for i in range(0, NITER - 1, 2):
                sup.append((i, i + 1))
            sup.append((NITER - 1,))

            for group in sup:
                fw = NT * len(group)
                up = ps.tile([72, 2 * NT], F32)
                h0p = ps.tile([128, 2 * NT], F32)
                h1p = ps.tile([128, 2 * NT], F32)
                yp = ps.tile([64, 2 * NT], F32)
                for j, i in enumerate(group):
                    nc.tensor.matmul(out=up[:, j * NT:(j + 1) * NT],
                                     lhsT=Rt, rhs=fpvts[i][:],
                                     start=True, stop=True)
                rt = io.tile([72, 2 * NT], F32)
                nc.vector.tensor_scalar(out=rt[:, 0:fw], in0=up[:, 0:fw],
                                        scalar1=CMAGIC, scalar2=CMAGIC,
                                        op0=ALU.add, op1=ALU.subtract)
                vt = io.tile([72, 2 * NT], F32)
                nc.vector.tensor_tensor(out=vt[:, 0:fw], in0=up[:, 0:fw],
                                        in1=rt[:, 0:fw], op=ALU.subtract)
                s36t = io.tile([72, 2 * NT], F32R)
                nc.scalar.activation(s36t[:, 0:fw], vt[:, 0:fw], AF.Sin,
                                     bias=0.0, scale=float(2 * np.pi))

                for j, i in enumerate(group):
                    js = slice(j * NT, (j + 1) * NT)
                    nc.tensor.matmul(out=h0p[:, js], lhsT=W0at,
                                     rhs=fpvts[i][:], start=True, stop=False)
                    nc.tensor.matmul(out=h0p[:, js], lhsT=W0st,
                                     rhs=s36t[:, js], start=False, stop=True)
                h0t = io.tile([128, 2 * NT], F32R)
                nc.scalar.activation(h0t[:, 0:fw], h0p[:, 0:fw], AF.Relu,
                                     bias=b0t, scale=1.0)

                for j, i in enumerate(group):
                    js = slice(j * NT, (j + 1) * NT)
                    nc.tensor.matmul(out=h1p[:, js], lhsT=W1t,
                                     rhs=h0t[:, js], start=True, stop=True)
                h1t = io.tile([128, 2 * NT], F32R)
                nc.scalar.activation(h1t[:, 0:fw], h1p[:, 0:fw], AF.Relu,
                                     bias=b1t, scale=1.0)

                for j, i in enumerate(group):
                    js = slice(j * NT, (j + 1) * NT)
                    nc.tensor.matmul(out=yp[:, js], lhsT=W2t,
                                     rhs=h1t[:, js], start=True, stop=True)
                yt = io.tile([64, 2 * NT], F32)
                nc.vector.tensor_scalar(out=yt[:, 0:fw], in0=yp[:, 0:fw],
                                        scalar1=b2t, scalar2=None,
                                        op0=ALU.add)
                i0 = group[0]
                nc.sync.dma_start(out=y_d[:, i0 * NT:i0 * NT + fw],
                                  in_=yt[:, 0:fw])

    nc.compile()
    return nc


def _get_program():
    global _PREP
    if _PREP is None:
        _PREP = _build_program()
    return _PREP


def _pack_weights(W0, b0, W1, b1, W2, b2):
    """Per-expert consolidated [128, _COLS] device weight array."""
    W0a = np.zeros((39, 64), np.float32)
    W0a[0:38] = W0[_W0A_ROWS]
    W0s = W0[_W0S_ROWS].astype(np.float32)

    Rp = _build_R()
    wall = np.zeros((128, _COLS), np.float32)
    wall[32:39, 0:36] = Rp          # Rdd block A
    wall[71:78, 36:72] = Rp         # Rdd block B
    wall[0:39, 72:136] = W0a        # W0add block A
    wall[39:78, 136:200] = W0a      # W0add block B
    wall[0:36, 200:264] = W0s       # W0sdd block A
    wall[36:72, 264:328] = W0s      # W0sdd block B
    wall[0:64, 328:392] = W1        # W1dd block A
    wall[64:128, 392:456] = W1      # W1dd block B
    wall[0:64, 456:488] = W2        # W2dd block A
    wall[64:128, 488:520] = W2      # W2dd block B
    wall[0:128, 520] = np.concatenate([b0, b0])
    wall[0:128, 521] = np.concatenate([b1, b1])
    wall[0:64, 522] = np.concatenate([b2, b2])
    return wall


def kernel(idxs, positions, viewdirs, features, W0, b0, W1, b1, W2, b2):
    from concourse.bass_utils import run_bass_kernel_spmd

    N = idxs.shape[0]
    idx = idxs.reshape(-1).astype(np.int64)
    out = np.zeros((N, D), np.float32)

    # Route: list of (expert, sample-index-array) chunks of <= C samples.
    chunks = []
    for k in range(K):
        sel = np.nonzero(idx == k)[0]
        for lo in range(0, len(sel), C):
            chunks.append((k, sel[lo:lo + C]))

    walls = [_pack_weights(W0[k], b0[k], W1[k], b1[k], W2[k], b2[k])
             for k in range(K)]

    nc = _get_program()
    zero_in = None
    for inv in range(0, len(chunks), 8):
        batch = chunks[inv:inv + 8]
        in_maps = []
        for ci in range(8):
            if ci < len(batch):
                k, sel = batch[ci]
                n = len(sel)
                fpv39 = np.zeros((39, C), np.float32)
                fpv39[0:32, :n] = features[sel].T
                fpv39[32:35, :n] = positions[sel].T
                fpv39[35:38, :n] = viewdirs[sel].T
                fpv39[38, :] = 1.0
                fpv78 = np.concatenate(
                    [fpv39.reshape(39, NITER, 2, NT)[:, :, 0],
                     fpv39.reshape(39, NITER, 2, NT)[:, :, 1]],
                    axis=0).reshape(78, C // 2)
                in_maps.append({"fpv": np.ascontiguousarray(fpv78),
                                "wall": walls[k]})
            else:
                if zero_in is None:
                    zero_in = {"fpv": np.zeros((78, C // 2), np.float32),
                               "wall": walls[0]}
                in_maps.append(zero_in)
        global _LAST_IN_MAPS
        _LAST_IN_MAPS = in_maps
        res = None
        for attempt in range(3):
            try:
                res = run_bass_kernel_spmd(nc, in_maps,
                                           core_ids=list(range(8)))
                break
            except Exception:
                if attempt == 2:
                    raise
        assert res is not None
        for ci, (k, sel) in enumerate(batch):
            y64 = res.results[ci]["y"]                   # [64, C//2]
            y32 = np.stack([y64[0:32].reshape(D, NITER, NT),
                            y64[32:64].reshape(D, NITER, NT)],
                           axis=2).reshape(D, C)
            out[sel] = y32[:, :len(sel)].T
    return out
